# revision 11
# baseline (speedup 1.0000x reference)
"""Centered-x GLU kernel.

Host pre-casts x to bf16 and folds the entmax mask into W2 (pair-tile
layout). Per 512-row superchunk (two virtual batches):
  - x^T arrives via DMA-transpose (bf16, no PE transposes),
  - per-vb mean of x is removed on the x^T side (TSP), so the BN shift
    reduces to beta and the matmul output is already centered,
  - main matmuls produce y^T tiles in PSUM; sigmoid half is read out by
    ACT (scale = gamma*rstd per (tile, vb), bias = beta), linear half by
    a single full-span ACT Relu; products on GPSIMD; the per-channel
    linear scale rides the fold matmul weights,
  - variance via per-vb Gram matrices: G = x^T x on PE, E[y^2] =
    diag(W^T G W) with the diagonal extracted by DVE STT+accum,
  - rsqrt via quake seed + 2 Newton iterations (stats batched over
    superchunk pairs to amortize small-op overhead).

Stats are emitted two superchunks ahead of the main path so the
per-engine in-order queues overlap stats and mains.
"""

import sys

import numpy as np

if "/opt/trn_rl_repo" not in sys.path:
    sys.path.insert(0, "/opt/trn_rl_repo")

import ml_dtypes

N_CORES = 8
B_FULL = 65536
B_CORE = B_FULL // N_CORES          # 8192
D_IN = 128
N_PATH = 8
C_TOT = 1024
VBS = 256
SUP = 512                            # superchunk rows (2 vb)
N_SUP = B_CORE // SUP                # 16
XBLK = 2048                          # rows per dma-transpose block
N_XBLK = B_CORE // XBLK              # 4
BN_EPS = 1e-5

bf16 = ml_dtypes.bfloat16

SIG = (0, 2, 4, 6)
LIN = (1, 3, 5, 7)


def _entmax15_np(x):
    """Exact entmax alpha=1.5 along last axis (numpy port of reference)."""
    x = np.asarray(x, np.float32)
    x = x - x.max(-1, keepdims=True)
    x = x / 2.0
    Xsrt = np.sort(x, -1)[..., ::-1].astype(np.float32)
    d = x.shape[-1]
    rho = np.arange(1, d + 1, dtype=np.float32)
    mean = np.cumsum(Xsrt, -1) / rho
    mean_sq = np.cumsum(Xsrt * Xsrt, -1) / rho
    ss = rho * (mean_sq - mean * mean)
    delta = np.clip((1.0 - ss) / rho, 0.0, None)
    tau = mean - np.sqrt(delta)
    support = (tau <= Xsrt).sum(-1, keepdims=True)
    tau_star = np.take_along_axis(tau, support - 1, axis=-1)
    return np.clip(x - tau_star, 0.0, None) ** 2


def _arrange_params(w2, gamma, beta):
    """Pair-tile layout: tile T holds (k=T//2, ab=T%2); partition j of
    tile T is channel (2k + j//64)*128 + ab*64 + (j%64). g32/b32 are
    [128, 32] with column T*4 + vv (vv duplicated)."""
    w2_arr = np.empty_like(w2)
    g32 = np.empty((128, 32), np.float32)
    b32 = np.empty((128, 32), np.float32)
    for T in range(8):
        k, ab = T // 2, T % 2
        for h in range(2):
            path = 2 * k + h
            cols = slice(path * 128 + ab * 64, path * 128 + ab * 64 + 64)
            w2_arr[:, T * 128 + h * 64: T * 128 + h * 64 + 64] = w2[:, cols]
            for vv in range(4):
                g32[h * 64:(h + 1) * 64, T * 4 + vv] = gamma[cols]
                b32[h * 64:(h + 1) * 64, T * 4 + vv] = beta[cols]
    return w2_arr, g32, b32


_BUILT = None
_BUILT_FAST = None


def _build_bass(lin_fast):
    import concourse.bacc as bacc
    import concourse.mybir as mybir
    from concourse.tile import TileContext
    from contextlib import ExitStack

    f32 = mybir.dt.float32
    bf = mybir.dt.bfloat16
    i32 = mybir.dt.int32
    AF = mybir.ActivationFunctionType
    OP = mybir.AluOpType

    nc = bacc.Bacc()

    x_d = nc.declare_dram_parameter("xb", [B_CORE, D_IN], bf, isOutput=False)
    w2_d = nc.declare_dram_parameter("w2", [D_IN, C_TOT], bf, isOutput=False)
    w2t_d = nc.declare_dram_parameter("w2t", [D_IN, C_TOT], bf, isOutput=False)
    g32_d = nc.declare_dram_parameter("g32", [128, 32], f32, isOutput=False)
    b32_d = nc.declare_dram_parameter("b32", [128, 32], f32, isOutput=False)
    fb_d = nc.declare_dram_parameter("fb", [128, 64], bf, isOutput=False)
    out_d = nc.declare_dram_parameter("out", [B_CORE, 64], f32, isOutput=True)

    with TileContext(nc) as tc, ExitStack() as es:
        cpool = es.enter_context(tc.tile_pool(name="consts", bufs=1))
        w2_sb = cpool.tile([128, C_TOT], bf, tag="w2")
        w2t_sb = cpool.tile([128, C_TOT], bf, tag="w2t")
        g32_sb = cpool.tile([128, 32], f32, tag="g32")
        b32_sb = cpool.tile([128, 32], f32, tag="b32")
        fb_sb = cpool.tile([128, 64], bf, tag="fb")
        one_sb = cpool.tile([128, 1], bf, tag="oneb")

        nc.vector.memset(one_sb[:], 1.0)

        # SBUF pools
        xbt_p = es.enter_context(tc.tile_pool(name="xbt", bufs=3))
        xin_p = es.enter_context(tc.tile_pool(name="xin", bufs=8))
        xcs_p = es.enter_context(tc.tile_pool(name="xcs", bufs=4))
        gt_p = es.enter_context(tc.tile_pool(name="gt", bufs=3))
        rt_p = es.enter_context(tc.tile_pool(name="rt", bufs=3))
        pr_p = es.enter_context(tc.tile_pool(name="pr", bufs=3))
        gs_p = es.enter_context(tc.tile_pool(name="gs", bufs=4))
        st_p = es.enter_context(tc.tile_pool(name="st", bufs=4))
        fw_p = es.enter_context(tc.tile_pool(name="fw", bufs=12))
        ot_p = es.enter_context(tc.tile_pool(name="ot", bufs=3))
        scr_p = es.enter_context(tc.tile_pool(name="scr", bufs=2))

        # PSUM banks (8): yps 2x2, hp 2x1, mis 1 (mu/muy + G region), otp 1
        yps_p = es.enter_context(tc.tile_pool(name="yps", bufs=2, space="PSUM"))
        hp_p = es.enter_context(tc.tile_pool(name="hp", bufs=2, space="PSUM"))
        mis_p = es.enter_context(tc.tile_pool(name="mis", bufs=1, space="PSUM"))
        otp_p = es.enter_context(tc.tile_pool(name="otp", bufs=1, space="PSUM"))

        # natural layout: row = s*512 + t*128 + p  (t=0..3; vb u = t//2)
        x_r = x_d[:, :].rearrange("(s t p) d -> s p t d", p=128, t=4)
        # transposed blocks: 2048 rows each
        x_flat = x_d[:, :].rearrange("(b r) d -> b r d", r=XBLK)
        out_r = out_d[:, :].rearrange("(s t p) o -> s p t o", p=128, t=4)

        # PE warmups to absorb const-DMA semaphores / pstate ramp
        warm = hp_p.tile([128, 2, 128], f32, tag="hp", name="warm")
        nc.tensor.matmul(warm[:, 0, :], w2_sb[:, 0:128], w2_sb[:, 0:128],
                         start=True, stop=True)
        warm2 = hp_p.tile([128, 2, 128], f32, tag="hp", name="warm2")
        nc.tensor.matmul(warm2[:, 0, :], w2t_sb[:, 0:128], w2t_sb[:, 0:128],
                         start=True, stop=True)

        pair_tiles = {}
        xbts = {}
        consts_emitted = []

        def emit_consts():
            nc.sync.dma_start(out=w2_sb[:], in_=w2_d[:, :])
            nc.sync.dma_start(out=w2t_sb[:], in_=w2t_d[:, :])
            nc.sync.dma_start(out=g32_sb[:], in_=g32_d[:, :])
            nc.sync.dma_start(out=b32_sb[:], in_=b32_d[:, :])
            nc.sync.dma_start(out=fb_sb[:], in_=fb_d[:, :])

        def emit_xbt(b):
            xbt = xbt_p.tile([128, XBLK], bf, tag="xbt", name=f"xbt{b}")
            nc.sync.dma_start_transpose(xbt[:], x_flat[b])
            return xbt

        def emit_xin(sp):
            xin = xin_p.tile([128, 4, 128], bf, tag="xin", name=f"xin{sp}")
            nc.sync.dma_start(out=xin[:], in_=x_r[sp])
            pair_tiles.setdefault(("xin", sp), xin)
            return xin

        def emit_gram(sp):
            """G + mean matmuls for superchunk sp (xin loaded a step ago)."""
            P, h = sp // 2, sp % 2
            if h == 0:
                # cols 0:4 mu sums, 4:36 muy, 36:292 the per-superchunk G
                mis = mis_p.tile([128, 292], f32, tag="mis", name=f"mis{P}")
                ss = st_p.tile([128, 32], f32, tag="ss", name=f"ss{P}")
                pair_tiles[P] = {"mis": mis, "ss": ss}
            mis = pair_tiles[P]["mis"]
            xin = pair_tiles[("xin", sp)]
            for u in range(2):
                for j, t in enumerate((2 * u, 2 * u + 1)):
                    nc.tensor.matmul(mis[:, 36 + u * 128: 36 + (u + 1) * 128],
                                     xin[:, t, :], xin[:, t, :],
                                     start=(j == 0), stop=(j == 1))
            for u in range(2):
                vv = h * 2 + u
                for j, t in enumerate((2 * u, 2 * u + 1)):
                    nc.tensor.matmul(mis[:, vv:vv + 1], xin[:, t, :],
                                     one_sb[:], start=(j == 0), stop=(j == 1))

        def emit_gsb(sp):
            """Gram readout + means for sp (G done a step ago)."""
            P, h = sp // 2, sp % 2
            mis = pair_tiles[P]["mis"]
            gsb = gs_p.tile([128, 2, 128], bf, tag="gsb", name=f"gsb{sp}")
            nc.vector.tensor_copy(
                gsb[:], mis[:, 36:292].rearrange("p (v j) -> p v j", v=2))
            musf = st_p.tile([128, 2], f32, tag="musf", name=f"musf{sp}")
            nc.vector.tensor_scalar_mul(musf[:], mis[:, h * 2:h * 2 + 2],
                                        1.0 / VBS)
            musb = st_p.tile([128, 2], bf, tag="musb", name=f"musb{sp}")
            nc.vector.tensor_copy(musb[:], musf[:])
            pair_tiles[P][f"musf{h}"] = musf
            pair_tiles[P][f"musb{h}"] = musb
            pair_tiles[P][f"gsb{h}"] = gsb

        def emit_muy(sp):
            P, h = sp // 2, sp % 2
            mis = pair_tiles[P]["mis"]
            musb = pair_tiles[P][f"musb{h}"]
            for T in range(8):
                nc.tensor.matmul(
                    mis[:, 4 + T * 4 + h * 2: 6 + T * 4 + h * 2],
                    w2_sb[:, T * 128:(T + 1) * 128], musb[:],
                    start=True, stop=True)

        def emit_hp(sp):
            """Quadratic-form matmuls for sp (gsb from previous step)."""
            P, h = sp // 2, sp % 2
            gsb = pair_tiles[P][f"gsb{h}"]
            hps = []
            for g in range(2):
                hp = hp_p.tile([128, 2, 2, 128], f32, tag="hp",
                               name=f"hp{sp}_{g}")
                for i in range(2):
                    T = g * 2 + i
                    nc.tensor.matmul(
                        hp[:, i].rearrange("p v j -> p (v j)"),
                        w2_sb[:, T * 128:(T + 1) * 128],
                        gsb[:].rearrange("p v j -> p (v j)"),
                        start=True, stop=True)
                hps.append(hp)
            for g in range(2):
                hp = hp_p.tile([128, 2, 2, 128], f32, tag="hp",
                               name=f"hp{sp}_{2 + g}")
                for i in range(2):
                    T = 4 + g * 2 + i
                    nc.tensor.matmul(
                        hp[:, i].rearrange("p v j -> p (v j)"),
                        w2_sb[:, T * 128:(T + 1) * 128],
                        gsb[:].rearrange("p v j -> p (v j)"),
                        start=True, stop=True)
                hps.append(hp)
            pair_tiles[("hp", sp)] = hps

        def emit_diag(sp):
            """Diagonal extraction for sp (hp this step, earlier in PE order)."""
            P, h = sp // 2, sp % 2
            ss = pair_tiles[P]["ss"]
            hps = pair_tiles.pop(("hp", sp))
            for g in range(4):
                hp = hps[g]
                for i in range(2):
                    T = g * 2 + i
                    for u in range(2):
                        vv = h * 2 + u
                        scr = scr_p.tile([128, 128], bf, tag="scr",
                                         name=f"scr{sp}_{T}_{u}")
                        nc.vector.scalar_tensor_tensor(
                            scr[:], hp[:, i, u, :], 1.0 / VBS,
                            w2t_sb[:, T * 128:(T + 1) * 128],
                            OP.mult, OP.mult,
                            accum_out=ss[:, T * 4 + vv: T * 4 + vv + 1])

        def emit_stats_pairend(P):
            """var -> rstd -> scl (+ scaled fold weights) for pair P."""
            mis = pair_tiles[P]["mis"]
            ss = pair_tiles[P]["ss"]
            vpe = st_p.tile([128, 32], f32, tag="vpe", name=f"vpe{P}")
            msq = st_p.tile([128, 32], f32, tag="msq", name=f"msq{P}")
            nc.scalar.activation(msq[:], mis[:, 4:36], AF.Square)
            nc.vector.scalar_tensor_tensor(
                vpe[:], ss[:], BN_EPS, msq[:], OP.add, OP.subtract)
            rs = st_p.tile([128, 32], f32, tag="rs", name=f"rs{P}")
            nc.vector.tensor_scalar(rs[:].bitcast(i32), vpe[:].bitcast(i32),
                                    -0.5, 1597463007.0, OP.mult, OP.add)
            q_ = st_p.tile([128, 32], f32, tag="q", name=f"q{P}")
            for _ in range(1):
                nc.vector.tensor_mul(q_[:], rs[:], vpe[:])
                nc.vector.scalar_tensor_tensor(
                    q_[:], q_[:], -0.5, rs[:], OP.mult, OP.mult)
                nc.vector.scalar_tensor_tensor(
                    rs[:], q_[:], 1.5, rs[:], OP.add, OP.mult)
            scl = st_p.tile([128, 32], f32, tag="scl", name=f"scl{P}")
            nc.vector.tensor_mul(scl[:], rs[:], g32_sb[:])
            pair_tiles[P]["scl"] = scl

            fws = []
            if lin_fast:
                for vv in range(4):
                    fw = fw_p.tile([128, 4, 64], bf, tag="fw",
                                   name=f"fw{P}_{vv}")
                    for k in range(4):
                        nc.vector.tensor_scalar_mul(
                            fw[:, k, :], fb_sb[:],
                            scl[:, LIN[k] * 4 + vv: LIN[k] * 4 + vv + 1])
                    fws.append(fw)
            pair_tiles[P]["fws"] = fws

        def emit_main_super(s, xbt):
            """Main path for superchunk s (2 subchunks of 256 rows)."""
            P, h = s // 2, s % 2
            pt = pair_tiles[P]
            musf, scl, fws = pt[f"musf{h}"], pt["scl"], pt["fws"]
            otp = otp_p.tile([128, 4, 64], f32, tag="otp", name=f"otp{s}")
            col0 = s * SUP - (s // 4) * XBLK
            xcss, sigys, linys, gts, rts = [], [], [], [], []
            for u in range(2):
                xcs = xcs_p.tile([128, VBS], bf, tag="xcs",
                                 name=f"xcs{s}_{u}")
                nc.vector.tensor_scalar(
                    xcs[:], xbt[:, col0 + u * VBS: col0 + (u + 1) * VBS],
                    musf[:, u:u + 1], None, OP.subtract)
                xcss.append(xcs)
            for u in range(2):
                sigy = yps_p.tile([128, 4, VBS], f32, tag="yps",
                                  name=f"sigy{s}_{u}")
                for i, T in enumerate(SIG):
                    nc.tensor.matmul(sigy[:, i, :],
                                     w2_sb[:, T * 128:(T + 1) * 128],
                                     xcss[u][:], start=True, stop=True)
                sigys.append(sigy)
                gt = gt_p.tile([128, 4, VBS], bf, tag="gt", name=f"gt{s}_{u}")
                vv = h * 2 + u
                for i, T in enumerate(SIG):
                    nc.scalar.activation(
                        gt[:, i, :], sigy[:, i, :], AF.Sigmoid,
                        bias=b32_sb[:, T * 4 + vv: T * 4 + vv + 1],
                        scale=scl[:, T * 4 + vv: T * 4 + vv + 1])
                gts.append(gt)
            for u in range(2):
                vv = h * 2 + u
                liny = yps_p.tile([128, 4, VBS], f32, tag="yps",
                                  name=f"liny{s}_{u}")
                for i, T in enumerate(LIN):
                    nc.tensor.matmul(liny[:, i, :],
                                     w2_sb[:, T * 128:(T + 1) * 128],
                                     xcss[u][:], start=True, stop=True)
                rt = rt_p.tile([128, 4, VBS], bf, tag="rt", name=f"rt{s}_{u}")
                if lin_fast:
                    nc.scalar.activation(rt[:], liny[:], AF.Relu)
                else:
                    for i, T in enumerate(LIN):
                        nc.scalar.activation(
                            rt[:, i, :], liny[:, i, :], AF.Relu,
                            bias=b32_sb[:, T * 4 + vv: T * 4 + vv + 1],
                            scale=scl[:, T * 4 + vv: T * 4 + vv + 1])

                pr = pr_p.tile([128, 4, VBS], bf, tag="pr", name=f"pr{s}_{u}")
                nc.gpsimd.tensor_mul(pr[:], gts[u][:], rt[:])

                for tt in range(2):
                    for k in range(4):
                        nc.tensor.matmul(
                            otp[:, u * 2 + tt, :],
                            pr[:, k, tt * 128:(tt + 1) * 128],
                            fws[vv][:, k, :] if lin_fast else fb_sb[:],
                            start=(k == 0), stop=(k == 3))

            ots = ot_p.tile([128, 4, 64], f32, tag="ots", name=f"ots{s}")
            if s % 2 == 0:
                nc.vector.tensor_copy(ots[:], otp[:])
            else:
                nc.scalar.copy(ots[:], otp[:])
            nc.sync.dma_start(out=out_r[s], in_=ots[:])

        # software pipeline (per step t):
        #   DMA: xbt block / xin for t+4;  PE: G/mu(t+3);  DVE: xcs(t),
        #   diag(t+2), gsb/mus(t+3), pairend;  PE: hp(t+2) between the two
        #   subchunk matmul groups;  ACT/Pool/PE: mains(t).
        for t in range(-4, N_SUP):
            if t + 4 < N_SUP:
                emit_xin(t + 4)
                if not consts_emitted:
                    emit_consts()
                    consts_emitted.append(True)
                if (t + 4) % 4 == 0:
                    xbts[(t + 4) // 4] = emit_xbt((t + 4) // 4)
            if 0 <= t + 3 < N_SUP:
                emit_gram(t + 3)
            if 0 <= t:
                emit_main_super(t, xbts[t // 4])
            if 0 <= t + 2 < N_SUP:
                emit_hp(t + 2)
                emit_diag(t + 2)
                if (t + 2) % 2 == 1:
                    emit_stats_pairend((t + 2) // 2)
            if 0 <= t + 3 < N_SUP:
                emit_gsb(t + 3)
                emit_muy(t + 3)

    nc.compile()
    return nc


def kernel(x, mask_w, conv_w, conv_b, gamma, beta):
    global _BUILT, _BUILT_FAST
    from concourse.bass_utils import run_bass_kernel_spmd

    x = np.asarray(x, np.float32)
    mask = _entmax15_np(np.asarray(mask_w, np.float32))
    w2 = (np.asarray(conv_w, np.float32) * mask[:, None, :]).transpose(2, 0, 1)
    w2 = np.ascontiguousarray(w2.reshape(D_IN, C_TOT), np.float32)
    gamma = np.asarray(gamma, np.float32)
    beta = np.asarray(beta, np.float32)
    w2a, g32, b32 = _arrange_params(w2, gamma, beta)
    w2t = np.empty_like(w2a)
    for T in range(8):
        blk = w2a[:, T * 128:(T + 1) * 128]
        w2t[:, T * 128:(T + 1) * 128] = blk.T
    fb = np.zeros((128, 64), np.float32)
    fb[np.arange(128), np.arange(128) % 64] = 1.0

    # fast linear path: relu commutes with the (positive) scale and the
    # shift is zero after x-centering
    lin_cols = np.zeros(1024, bool)
    for T in LIN:
        k, ab = T // 2, T % 2
        for h in range(2):
            path = 2 * k + h
            lin_cols[path * 128 + ab * 64: path * 128 + ab * 64 + 64] = True
    lin_fast = bool(np.all(beta[lin_cols] == 0.0)
                    and np.all(gamma[lin_cols] >= 0.0))

    if _BUILT is None or _BUILT_FAST != lin_fast:
        _BUILT = _build_bass(lin_fast)
        _BUILT_FAST = lin_fast
    nc = _BUILT

    xb = x.astype(bf16)
    shards = xb.reshape(N_CORES, B_CORE, D_IN)
    in_maps = [
        {"xb": np.ascontiguousarray(shards[i]),
         "w2": np.ascontiguousarray(w2a.astype(bf16)),
         "w2t": np.ascontiguousarray(w2t.astype(bf16)),
         "g32": np.ascontiguousarray(g32),
         "b32": np.ascontiguousarray(b32),
         "fb": np.ascontiguousarray(fb.astype(bf16))}
        for i in range(N_CORES)
    ]
    res = run_bass_kernel_spmd(nc, in_maps, list(range(N_CORES)))
    return np.concatenate([res.results[i]["out"] for i in range(N_CORES)],
                          axis=0)


# revision 12
# speedup vs baseline: 1.0928x; 1.0928x over previous
"""Centered-x GLU kernel.

Host pre-casts x to bf16 and folds the entmax mask into W2 (pair-tile
layout). Per 512-row superchunk (two virtual batches):
  - x^T arrives via DMA-transpose (bf16, no PE transposes),
  - per-vb mean of x is removed on the x^T side (TSP), so the BN shift
    reduces to beta and the matmul output is already centered,
  - main matmuls produce y^T tiles in PSUM; sigmoid half is read out by
    ACT (scale = gamma*rstd per (tile, vb), bias = beta), linear half by
    a single full-span ACT Relu; products on GPSIMD; the per-channel
    linear scale rides the fold matmul weights,
  - variance via per-vb Gram matrices: G = x^T x on PE, E[y^2] =
    diag(W^T G W) with the diagonal extracted by DVE STT+accum,
  - rsqrt via quake seed + 2 Newton iterations (stats batched over
    superchunk pairs to amortize small-op overhead).

Stats are emitted two superchunks ahead of the main path so the
per-engine in-order queues overlap stats and mains.
"""

import sys

import numpy as np

if "/opt/trn_rl_repo" not in sys.path:
    sys.path.insert(0, "/opt/trn_rl_repo")

import ml_dtypes

N_CORES = 8
B_FULL = 65536
B_CORE = B_FULL // N_CORES          # 8192
D_IN = 128
N_PATH = 8
C_TOT = 1024
VBS = 256
SUP = 512                            # superchunk rows (2 vb)
N_SUP = B_CORE // SUP                # 16
XBLK = 2048                          # rows per dma-transpose block
N_XBLK = B_CORE // XBLK              # 4
BN_EPS = 1e-5

bf16 = ml_dtypes.bfloat16

SIG = (0, 2, 4, 6)
LIN = (1, 3, 5, 7)


def _entmax15_np(x):
    """Exact entmax alpha=1.5 along last axis (numpy port of reference)."""
    x = np.asarray(x, np.float32)
    x = x - x.max(-1, keepdims=True)
    x = x / 2.0
    Xsrt = np.sort(x, -1)[..., ::-1].astype(np.float32)
    d = x.shape[-1]
    rho = np.arange(1, d + 1, dtype=np.float32)
    mean = np.cumsum(Xsrt, -1) / rho
    mean_sq = np.cumsum(Xsrt * Xsrt, -1) / rho
    ss = rho * (mean_sq - mean * mean)
    delta = np.clip((1.0 - ss) / rho, 0.0, None)
    tau = mean - np.sqrt(delta)
    support = (tau <= Xsrt).sum(-1, keepdims=True)
    tau_star = np.take_along_axis(tau, support - 1, axis=-1)
    return np.clip(x - tau_star, 0.0, None) ** 2


def _arrange_params(w2, gamma, beta):
    """Pair-tile layout: tile T holds (k=T//2, ab=T%2); partition j of
    tile T is channel (2k + j//64)*128 + ab*64 + (j%64). g32/b32 are
    [128, 32] with column T*4 + vv (vv duplicated)."""
    w2_arr = np.empty_like(w2)
    g32 = np.empty((128, 32), np.float32)
    b32 = np.empty((128, 32), np.float32)
    for T in range(8):
        k, ab = T // 2, T % 2
        for h in range(2):
            path = 2 * k + h
            cols = slice(path * 128 + ab * 64, path * 128 + ab * 64 + 64)
            w2_arr[:, T * 128 + h * 64: T * 128 + h * 64 + 64] = w2[:, cols]
            for vv in range(4):
                g32[h * 64:(h + 1) * 64, T * 4 + vv] = gamma[cols]
                b32[h * 64:(h + 1) * 64, T * 4 + vv] = beta[cols]
    return w2_arr, g32, b32


_BUILT = None
_BUILT_FAST = None


def _build_bass(lin_fast):
    import concourse.bacc as bacc
    import concourse.mybir as mybir
    from concourse.tile import TileContext
    from contextlib import ExitStack

    f32 = mybir.dt.float32
    bf = mybir.dt.bfloat16
    i32 = mybir.dt.int32
    AF = mybir.ActivationFunctionType
    OP = mybir.AluOpType

    nc = bacc.Bacc()

    x_d = nc.declare_dram_parameter("xb", [B_CORE, D_IN], bf, isOutput=False)
    w2_d = nc.declare_dram_parameter("w2", [D_IN, C_TOT], bf, isOutput=False)
    w2t_d = nc.declare_dram_parameter("w2t", [D_IN, C_TOT], bf, isOutput=False)
    g32_d = nc.declare_dram_parameter("g32", [128, 32], f32, isOutput=False)
    b32_d = nc.declare_dram_parameter("b32", [128, 32], f32, isOutput=False)
    fb_d = nc.declare_dram_parameter("fb", [128, 64], bf, isOutput=False)
    out_d = nc.declare_dram_parameter("out", [B_CORE, 64], f32, isOutput=True)

    with TileContext(nc) as tc, ExitStack() as es:
        cpool = es.enter_context(tc.tile_pool(name="consts", bufs=1))
        w2_sb = cpool.tile([128, C_TOT], bf, tag="w2")
        w2t_sb = cpool.tile([128, C_TOT], bf, tag="w2t")
        g32_sb = cpool.tile([128, 32], f32, tag="g32")
        b32_sb = cpool.tile([128, 32], f32, tag="b32")
        fb_sb = cpool.tile([128, 64], bf, tag="fb")
        one_sb = cpool.tile([128, 1], bf, tag="oneb")

        nc.vector.memset(one_sb[:], 1.0)

        # SBUF pools
        xbt_p = es.enter_context(tc.tile_pool(name="xbt", bufs=3))
        xin_p = es.enter_context(tc.tile_pool(name="xin", bufs=8))
        xcs_p = es.enter_context(tc.tile_pool(name="xcs", bufs=4))
        gt_p = es.enter_context(tc.tile_pool(name="gt", bufs=3))
        rt_p = es.enter_context(tc.tile_pool(name="rt", bufs=3))
        pr_p = es.enter_context(tc.tile_pool(name="pr", bufs=3))
        gs_p = es.enter_context(tc.tile_pool(name="gs", bufs=4))
        st_p = es.enter_context(tc.tile_pool(name="st", bufs=4))
        fw_p = es.enter_context(tc.tile_pool(name="fw", bufs=12))
        ot_p = es.enter_context(tc.tile_pool(name="ot", bufs=3))
        scr_p = es.enter_context(tc.tile_pool(name="scr", bufs=2))

        # PSUM banks (8): yps 2x2, hp 2x1, mis 1 (mu/muy + G region), otp 1
        yps_p = es.enter_context(tc.tile_pool(name="yps", bufs=2, space="PSUM"))
        hp_p = es.enter_context(tc.tile_pool(name="hp", bufs=2, space="PSUM"))
        mis_p = es.enter_context(tc.tile_pool(name="mis", bufs=1, space="PSUM"))
        otp_p = es.enter_context(tc.tile_pool(name="otp", bufs=1, space="PSUM"))

        # natural layout: row = s*512 + t*128 + p  (t=0..3; vb u = t//2)
        x_r = x_d[:, :].rearrange("(s t p) d -> s p t d", p=128, t=4)
        # transposed blocks: 2048 rows each
        x_flat = x_d[:, :].rearrange("(b r) d -> b r d", r=XBLK)
        out_r = out_d[:, :].rearrange("(s t p) o -> s p t o", p=128, t=4)

        # PE warmups to absorb const-DMA semaphores / pstate ramp
        warm = hp_p.tile([128, 2, 128], f32, tag="hp", name="warm")
        nc.tensor.matmul(warm[:, 0, :], w2_sb[:, 0:128], w2_sb[:, 0:128],
                         start=True, stop=True)
        warm2 = hp_p.tile([128, 2, 128], f32, tag="hp", name="warm2")
        nc.tensor.matmul(warm2[:, 0, :], w2t_sb[:, 0:128], w2t_sb[:, 0:128],
                         start=True, stop=True)

        pair_tiles = {}
        xbts = {}
        consts_emitted = []

        def emit_consts():
            nc.sync.dma_start(out=w2_sb[:], in_=w2_d[:, :])
            nc.sync.dma_start(out=w2t_sb[:], in_=w2t_d[:, :])
            nc.sync.dma_start(out=g32_sb[:], in_=g32_d[:, :])
            nc.sync.dma_start(out=b32_sb[:], in_=b32_d[:, :])
            nc.sync.dma_start(out=fb_sb[:], in_=fb_d[:, :])

        def emit_xbt(b):
            xbt = xbt_p.tile([128, XBLK], bf, tag="xbt", name=f"xbt{b}")
            nc.sync.dma_start_transpose(xbt[:], x_flat[b])
            return xbt

        def emit_xin(sp):
            xin = xin_p.tile([128, 4, 128], bf, tag="xin", name=f"xin{sp}")
            nc.sync.dma_start(out=xin[:], in_=x_r[sp])
            pair_tiles.setdefault(("xin", sp), xin)
            return xin

        def emit_gram(sp):
            """G + mean matmuls for superchunk sp (xin loaded a step ago)."""
            P, h = sp // 2, sp % 2
            if h == 0:
                # cols 0:4 mu sums, 4:36 muy, 36:292 the per-superchunk G
                mis = mis_p.tile([128, 292], f32, tag="mis", name=f"mis{P}")
                ss = st_p.tile([128, 32], f32, tag="ss", name=f"ss{P}")
                pair_tiles[P] = {"mis": mis, "ss": ss}
            mis = pair_tiles[P]["mis"]
            xin = pair_tiles[("xin", sp)]
            for u in range(2):
                for j, t in enumerate((2 * u, 2 * u + 1)):
                    nc.tensor.matmul(mis[:, 36 + u * 128: 36 + (u + 1) * 128],
                                     xin[:, t, :], xin[:, t, :],
                                     start=(j == 0), stop=(j == 1))
            for u in range(2):
                vv = h * 2 + u
                for j, t in enumerate((2 * u, 2 * u + 1)):
                    nc.tensor.matmul(mis[:, vv:vv + 1], xin[:, t, :],
                                     one_sb[:], start=(j == 0), stop=(j == 1))

        def emit_gsb(sp):
            """Gram readout + means for sp (G done a step ago)."""
            P, h = sp // 2, sp % 2
            mis = pair_tiles[P]["mis"]
            gsb = gs_p.tile([128, 2, 128], bf, tag="gsb", name=f"gsb{sp}")
            nc.vector.tensor_copy(
                gsb[:], mis[:, 36:292].rearrange("p (v j) -> p v j", v=2))
            musf = st_p.tile([128, 2], f32, tag="musf", name=f"musf{sp}")
            nc.vector.tensor_scalar_mul(musf[:], mis[:, h * 2:h * 2 + 2],
                                        1.0 / VBS)
            musb = st_p.tile([128, 2], bf, tag="musb", name=f"musb{sp}")
            nc.vector.tensor_copy(musb[:], musf[:])
            pair_tiles[P][f"musf{h}"] = musf
            pair_tiles[P][f"musb{h}"] = musb
            pair_tiles[P][f"gsb{h}"] = gsb

        def emit_muy(sp):
            P, h = sp // 2, sp % 2
            mis = pair_tiles[P]["mis"]
            musb = pair_tiles[P][f"musb{h}"]
            for T in range(8):
                nc.tensor.matmul(
                    mis[:, 4 + T * 4 + h * 2: 6 + T * 4 + h * 2],
                    w2_sb[:, T * 128:(T + 1) * 128], musb[:],
                    start=True, stop=True)

        def emit_hp(sp):
            """Quadratic-form matmuls for sp (gsb from previous step)."""
            P, h = sp // 2, sp % 2
            gsb = pair_tiles[P][f"gsb{h}"]
            hps = []
            for g in range(2):
                hp = hp_p.tile([128, 2, 2, 128], f32, tag="hp",
                               name=f"hp{sp}_{g}")
                for i in range(2):
                    T = g * 2 + i
                    nc.tensor.matmul(
                        hp[:, i].rearrange("p v j -> p (v j)"),
                        w2_sb[:, T * 128:(T + 1) * 128],
                        gsb[:].rearrange("p v j -> p (v j)"),
                        start=True, stop=True)
                hps.append(hp)
            for g in range(2):
                hp = hp_p.tile([128, 2, 2, 128], f32, tag="hp",
                               name=f"hp{sp}_{2 + g}")
                for i in range(2):
                    T = 4 + g * 2 + i
                    nc.tensor.matmul(
                        hp[:, i].rearrange("p v j -> p (v j)"),
                        w2_sb[:, T * 128:(T + 1) * 128],
                        gsb[:].rearrange("p v j -> p (v j)"),
                        start=True, stop=True)
                hps.append(hp)
            pair_tiles[("hp", sp)] = hps

        def emit_diag(sp):
            """Diagonal extraction for sp (hp this step, earlier in PE order)."""
            P, h = sp // 2, sp % 2
            ss = pair_tiles[P]["ss"]
            hps = pair_tiles.pop(("hp", sp))
            for g in range(4):
                hp = hps[g]
                for i in range(2):
                    T = g * 2 + i
                    for u in range(2):
                        vv = h * 2 + u
                        scr = scr_p.tile([128, 128], bf, tag="scr",
                                         name=f"scr{sp}_{T}_{u}")
                        nc.vector.scalar_tensor_tensor(
                            scr[:], hp[:, i, u, :], 1.0 / VBS,
                            w2t_sb[:, T * 128:(T + 1) * 128],
                            OP.mult, OP.mult,
                            accum_out=ss[:, T * 4 + vv: T * 4 + vv + 1])

        def emit_stats_pairend(P):
            """var -> rstd -> scl (+ scaled fold weights) for pair P."""
            mis = pair_tiles[P]["mis"]
            ss = pair_tiles[P]["ss"]
            vpe = st_p.tile([128, 32], f32, tag="vpe", name=f"vpe{P}")
            msq = st_p.tile([128, 32], f32, tag="msq", name=f"msq{P}")
            nc.scalar.activation(msq[:], mis[:, 4:36], AF.Square)
            nc.vector.scalar_tensor_tensor(
                vpe[:], ss[:], BN_EPS, msq[:], OP.add, OP.subtract)
            rs = st_p.tile([128, 32], f32, tag="rs", name=f"rs{P}")
            nc.vector.tensor_scalar(rs[:].bitcast(i32), vpe[:].bitcast(i32),
                                    -0.5, 1597463007.0, OP.mult, OP.add)
            q_ = st_p.tile([128, 32], f32, tag="q", name=f"q{P}")
            for _ in range(1):
                nc.vector.tensor_mul(q_[:], rs[:], vpe[:])
                nc.vector.scalar_tensor_tensor(
                    q_[:], q_[:], -0.5, rs[:], OP.mult, OP.mult)
                nc.vector.scalar_tensor_tensor(
                    rs[:], q_[:], 1.5, rs[:], OP.add, OP.mult)
            scl = st_p.tile([128, 32], f32, tag="scl", name=f"scl{P}")
            nc.vector.tensor_mul(scl[:], rs[:], g32_sb[:])
            pair_tiles[P]["scl"] = scl

            fws = []
            if lin_fast:
                for vv in range(4):
                    fw = fw_p.tile([128, 4, 64], bf, tag="fw",
                                   name=f"fw{P}_{vv}")
                    for k in range(4):
                        nc.vector.tensor_scalar_mul(
                            fw[:, k, :], fb_sb[:],
                            scl[:, LIN[k] * 4 + vv: LIN[k] * 4 + vv + 1])
                    fws.append(fw)
            pair_tiles[P]["fws"] = fws

        def emit_main_super(s, xbt):
            """Main path for superchunk s (2 subchunks of 256 rows)."""
            P, h = s // 2, s % 2
            pt = pair_tiles[P]
            musf, scl, fws = pt[f"musf{h}"], pt["scl"], pt["fws"]
            otp = otp_p.tile([128, 4, 64], f32, tag="otp", name=f"otp{s}")
            col0 = s * SUP - (s // 4) * XBLK
            for u in range(2):
                vv = h * 2 + u
                xcs = xcs_p.tile([128, VBS], bf, tag="xcs",
                                 name=f"xcs{s}_{u}")
                nc.vector.tensor_scalar(
                    xcs[:], xbt[:, col0 + u * VBS: col0 + (u + 1) * VBS],
                    musf[:, u:u + 1], None, OP.subtract)

                sigy = yps_p.tile([128, 4, VBS], f32, tag="yps",
                                  name=f"sigy{s}_{u}")
                for i, T in enumerate(SIG):
                    nc.tensor.matmul(sigy[:, i, :],
                                     w2_sb[:, T * 128:(T + 1) * 128],
                                     xcs[:], start=True, stop=True)
                liny = yps_p.tile([128, 4, VBS], f32, tag="yps",
                                  name=f"liny{s}_{u}")
                for i, T in enumerate(LIN):
                    nc.tensor.matmul(liny[:, i, :],
                                     w2_sb[:, T * 128:(T + 1) * 128],
                                     xcs[:], start=True, stop=True)

                gt = gt_p.tile([128, 4, VBS], bf, tag="gt", name=f"gt{s}_{u}")
                for i, T in enumerate(SIG):
                    nc.scalar.activation(
                        gt[:, i, :], sigy[:, i, :], AF.Sigmoid,
                        bias=b32_sb[:, T * 4 + vv: T * 4 + vv + 1],
                        scale=scl[:, T * 4 + vv: T * 4 + vv + 1])

                rt = rt_p.tile([128, 4, VBS], bf, tag="rt", name=f"rt{s}_{u}")
                if lin_fast:
                    nc.scalar.activation(rt[:], liny[:], AF.Relu)
                else:
                    for i, T in enumerate(LIN):
                        nc.scalar.activation(
                            rt[:, i, :], liny[:, i, :], AF.Relu,
                            bias=b32_sb[:, T * 4 + vv: T * 4 + vv + 1],
                            scale=scl[:, T * 4 + vv: T * 4 + vv + 1])

                pr = pr_p.tile([128, 4, VBS], bf, tag="pr", name=f"pr{s}_{u}")
                nc.gpsimd.tensor_mul(pr[:], gt[:], rt[:])

                for tt in range(2):
                    for k in range(4):
                        nc.tensor.matmul(
                            otp[:, u * 2 + tt, :],
                            pr[:, k, tt * 128:(tt + 1) * 128],
                            fws[vv][:, k, :] if lin_fast else fb_sb[:],
                            start=(k == 0), stop=(k == 3))

            ots = ot_p.tile([128, 4, 64], f32, tag="ots", name=f"ots{s}")
            if s % 2 == 0:
                nc.vector.tensor_copy(ots[:], otp[:])
            else:
                nc.scalar.copy(ots[:], otp[:])
            nc.sync.dma_start(out=out_r[s], in_=ots[:])

        # software pipeline (per step t):
        #   DMA: xbt block / xin for t+4;  PE: G/mu(t+3);  DVE: xcs(t),
        #   diag(t+2), gsb/mus(t+3), pairend;  PE: hp(t+2) between the two
        #   subchunk matmul groups;  ACT/Pool/PE: mains(t).
        for t in range(-4, N_SUP):
            if t + 4 < N_SUP:
                emit_xin(t + 4)
                if not consts_emitted:
                    emit_consts()
                    consts_emitted.append(True)
                if (t + 4) % 4 == 0:
                    xbts[(t + 4) // 4] = emit_xbt((t + 4) // 4)
            if 0 <= t + 3 < N_SUP:
                emit_gram(t + 3)
            if 0 <= t:
                emit_main_super(t, xbts[t // 4])
            if 0 <= t + 2 < N_SUP:
                emit_hp(t + 2)
                emit_diag(t + 2)
                if (t + 2) % 2 == 1:
                    emit_stats_pairend((t + 2) // 2)
            if 0 <= t + 3 < N_SUP:
                emit_gsb(t + 3)
                emit_muy(t + 3)

    nc.compile()
    return nc


def kernel(x, mask_w, conv_w, conv_b, gamma, beta):
    global _BUILT, _BUILT_FAST
    from concourse.bass_utils import run_bass_kernel_spmd

    x = np.asarray(x, np.float32)
    mask = _entmax15_np(np.asarray(mask_w, np.float32))
    w2 = (np.asarray(conv_w, np.float32) * mask[:, None, :]).transpose(2, 0, 1)
    w2 = np.ascontiguousarray(w2.reshape(D_IN, C_TOT), np.float32)
    gamma = np.asarray(gamma, np.float32)
    beta = np.asarray(beta, np.float32)
    w2a, g32, b32 = _arrange_params(w2, gamma, beta)
    w2t = np.empty_like(w2a)
    for T in range(8):
        blk = w2a[:, T * 128:(T + 1) * 128]
        w2t[:, T * 128:(T + 1) * 128] = blk.T
    fb = np.zeros((128, 64), np.float32)
    fb[np.arange(128), np.arange(128) % 64] = 1.0

    # fast linear path: relu commutes with the (positive) scale and the
    # shift is zero after x-centering
    lin_cols = np.zeros(1024, bool)
    for T in LIN:
        k, ab = T // 2, T % 2
        for h in range(2):
            path = 2 * k + h
            lin_cols[path * 128 + ab * 64: path * 128 + ab * 64 + 64] = True
    lin_fast = bool(np.all(beta[lin_cols] == 0.0)
                    and np.all(gamma[lin_cols] >= 0.0))

    if _BUILT is None or _BUILT_FAST != lin_fast:
        _BUILT = _build_bass(lin_fast)
        _BUILT_FAST = lin_fast
    nc = _BUILT

    xb = x.astype(bf16)
    shards = xb.reshape(N_CORES, B_CORE, D_IN)
    in_maps = [
        {"xb": np.ascontiguousarray(shards[i]),
         "w2": np.ascontiguousarray(w2a.astype(bf16)),
         "w2t": np.ascontiguousarray(w2t.astype(bf16)),
         "g32": np.ascontiguousarray(g32),
         "b32": np.ascontiguousarray(b32),
         "fb": np.ascontiguousarray(fb.astype(bf16))}
        for i in range(N_CORES)
    ]
    res = run_bass_kernel_spmd(nc, in_maps, list(range(N_CORES)))
    return np.concatenate([res.results[i]["out"] for i in range(N_CORES)],
                          axis=0)
